# revision 4
# baseline (speedup 1.0000x reference)
"""Trainium2 Bass kernel for nn_Attention_2284922602161 (linear attention).

Math per batch element b (C=512, Cq=64, N=4096):
    Q = Wq@x + bq            [Cq, N]
    K = Wk@x + bk            [Cq, N]
    Qn = Q / ||Q||_col ; Kn = K / ||K||_col      (L2 over channel dim per position)
    ksum = sum_n Kn + eps    [Cq]
    tailor[n] = 1 / (N + Qn[:,n].ksum)
    out[:,n] = gamma * tailor[n] * (V.sum(-1) + (Kn V^T)^T @ Qn[:,n]),  V = Wv@x+bv

KEY ALGEBRA (V is never materialized):
    A_aug := [Kn; 1] @ x^T                [65, 512]   (row 64 = xsum)
    M1_aug = A_aug @ Wv^T + [ksum; N] (x) bv          (rank-1 V-bias fold)
           = [Kn@V^T ; vsum^T]            [65, 512]
    out[:, n] = M1_aug^T @ [gamma*tailor*Qn ; gamma*tailor][:, n]
This removes the [512x512] V projection (70% of the naive PE work) and all
of its PSUM eviction traffic.

Sharding: pure data-parallel, one batch element per NeuronCore (B=8 over 8).

On-chip strategy (per core):
  - x is DMA'd ONCE as bf16 channel-major tiles [c'=128, n] (4MB instead of
    8MB fp32). Per n-tile: 4 PE transposes produce the position-major copy
    x^T [n, c] needed by the A_aug contraction (contraction over n requires
    n on partitions); both layouts are consumed from SBUF. Weight/const DMAs
    that are only needed after phase 1 are issued between x chunks.
  - pass A: Q^T|K^T psum [n, 128] via 4 bf16 matmuls (1 cyc/row) plus a
    K=1 ones-row matmul that folds the biases into PSUM directly.
  - K-norm: ACT square+accum reads PSUM, Sqrt + DVE reciprocal; Kn^T is
    evicted straight to bf16 by the Pool engine (phase-2 lhsT dtype). Q^T
    is evicted fp32 by Pool for the later tailor/qs math.
  - phase 2 (3 tiles behind): at_ps[c-slice] += x^T_slice^T @ [Kn^T|1]
    (4 quadrant groups [128, 65] in one PSUM bank) and ksrow [1, 65] via a
    ones-column matmul; yields A_aug^T and [ksum|N] with no transposes.
  - fold: M1_aug = A_aug@Wv^T (4 f32r matmuls, ap=512) + rank-1 bias fold,
    accumulated in one PSUM bank, evicted to bf16.
  - pass C: per-position scalars batched in halves (DVE), the second half
    emitted under the pass D/E pipeline.
  - pass D/E software-pipelined per 512-col group: transposes for group g+1
    are emitted before the pass-E matmuls of group g so the PE never waits
    on the qf eviction; evictions convert to bf16 and one batched DMA per
    group writes [4x128, 512] to DRAM.
Output DRAM is bf16 (halves the write traffic); host upcasts to fp32.
"""

import numpy as np
import ml_dtypes

B, C, H, W = 8, 512, 64, 64
N = H * W              # 4096
CQ = 64
P = 128
NT = N // P            # 32 n-tiles
KT = C // P            # 4 contraction tiles
EPS = 1e-6
NCHUNK = 512
XCHUNKS = (1, 3) + (4,) * 7    # n-tiles per x DMA (small first: PE starts early)
LAG = 6                # phase-2 pipeline slack (tiles)

BF16 = ml_dtypes.bfloat16

_CACHE = {}

# bf16 const blob columns: [identb 128 | onescol 1 | ones32 32 | biasqk 128 | onesrow 128]
_OCOL = P
_O32 = P + 1
_OBIAS = P + 1 + NT
_OROW = P + 1 + NT + 2 * CQ
_OONE = P + 1 + NT + 2 * CQ + P
_OBV = P + 1 + NT + 2 * CQ + P + P
_BLOBB_W = P + 1 + NT + 2 * CQ + P + P + C


def _build():
    import concourse.bacc as bacc
    import concourse.mybir as mybir
    import concourse.tile as tile
    import concourse.bass as bass
    from contextlib import ExitStack

    f32 = mybir.dt.float32
    f32r = mybir.dt.float32r
    bf16 = mybir.dt.bfloat16
    AF = mybir.ActivationFunctionType

    nc = bacc.Bacc("TRN2", target_bir_lowering=False, debug=False,
                   enable_asserts=True, num_devices=8)

    # x channel-major: x_d[p, i, k, j] = x[k*128+p, i*128+j]
    x_d = nc.declare_dram_parameter("x", [P, NT, KT, P], bf16, isOutput=False)
    wqk_d = nc.declare_dram_parameter("wqk", [P, KT, 2 * CQ], bf16,
                                      isOutput=False)
    wv_d = nc.declare_dram_parameter("wv", [P, KT, C], bf16, isOutput=False)
    bb_d = nc.declare_dram_parameter("blobb", [P, _BLOBB_W], bf16,
                                     isOutput=False)
    out_d = nc.declare_dram_parameter("out", [N, C], bf16, isOutput=True)

    with tile.TileContext(nc) as tc:
        with ExitStack() as ctx:
            const = ctx.enter_context(tc.tile_pool(name="const", bufs=1))
            xpool = ctx.enter_context(tc.tile_pool(name="xpool", bufs=3))
            xpmp = ctx.enter_context(tc.tile_pool(name="xpmp", bufs=LAG + 2))
            scp = ctx.enter_context(tc.tile_pool(name="scp", bufs=6))
            qfp = ctx.enter_context(tc.tile_pool(name="qfp", bufs=8))
            obp = ctx.enter_context(tc.tile_pool(name="obp", bufs=5))

            x_ap = x_d.ap()

            # --- early consts + first x chunk (late consts go between
            # x chunks inside the loop: they are not needed until fold) ---
            blobb = const.tile([P, _BLOBB_W], bf16)
            nc.sync.dma_start(out=blobb, in_=bb_d.ap())
            x_first = xpool.tile([P, XCHUNKS[0], KT, P], bf16,
                                 name="x_0", tag="x")
            nc.sync.dma_start(out=x_first, in_=x_ap[:, 0:XCHUNKS[0]])
            wqk_sb = const.tile([P, KT, 2 * CQ], bf16)
            nc.sync.dma_start(out=wqk_sb, in_=wqk_d.ap())
            wv_sb = const.tile([P, KT, C], bf16)

            identb = blobb[:, 0:P]
            onescol = blobb[:, _OCOL:_OCOL + 1]
            ones32 = blobb[:, _O32:_O32 + NT]
            biasqk = blobb[0:1, _OBIAS:_OBIAS + 2 * CQ]
            onesrow_b = blobb[0:1, _OROW:_OROW + P]
            ones128 = blobb[:, _OONE:_OONE + P]
            bvrow = blobb[0:1, _OBV:_OBV + C]

            # --- persistent per-batch buffers ---
            qraw_q = [const.tile([P, 8, CQ + 1], bf16, name=f"qraw_{q}")
                      for q in range(4)]            # [Q^T | ||Q||] per quarter
            kn_all = const.tile([P, NT, CQ + 1], bf16)    # [Kn^T | 1]
            qprod = const.tile([P, NT, CQ], f32)
            q_ss = const.tile([P, NT], f32)
            qrt_all = const.tile([P, NT], f32)
            qrs_all = const.tile([P, NT], f32)
            ataug_sb = const.tile([P, KT, CQ + 1], bf16)
            ksrow_sb = const.tile([1, CQ + 1], bf16)
            ks_eps = const.tile([P, CQ], f32)
            m1_sb = const.tile([CQ + 1, C], bf16)
            qd = const.tile([P, NT], f32)
            dn = const.tile([P, NT], f32)
            tailor = const.tile([P, NT], f32)
            aug_all = const.tile([P, NT], f32)
            s_all = const.tile([P, NT], f32)

            # aug ones column for all kn tiles in one strided write
            nc.vector.tensor_copy(out=kn_all[:, :, CQ], in_=ones32)

            psT_pool = ctx.enter_context(
                tc.tile_pool(name="psT", bufs=1, space="PSUM"))
            with tc.tile_pool(name="psAT", bufs=1, space="PSUM") as psAT_pool:
              with tc.tile_pool(name="psX", bufs=2, space="PSUM") as psX_pool, \
                   tc.tile_pool(name="psA", bufs=4, space="PSUM") as psA_pool:
                atks_ps = psAT_pool.tile([P, KT + 1, CQ + 1], f32)
                at_ps = atks_ps[:, 0:KT, :]
                ks_ps = atks_ps[:, KT, :]
                # HW psum start-zeroing is bank-granular: 5 regions share
                # this bank, so zero once and accumulate with start=False.
                nc.vector.memset(atks_ps, 0.0)

                pending = []
                xpms = {}
                x_g = None
                x_base = 0
                xc = 0
                qfs = {}
                tq = []     # groups whose qraw quarter is complete

                def emit_group_T(g):
                    tr_ps = psT_pool.tile([CQ + 1, 4, P], bf16,
                                          name=f"tr_{g}", tag="tr")
                    for u in range(4):
                        t = g * 4 + u
                        nc.tensor.transpose(
                            tr_ps[:, u, :],
                            qraw_q[t // 8][:, t % 8, :], identb)
                    qf = qfp.tile([CQ + 1, NCHUNK], bf16,
                                  name=f"qf_{g}", tag="qf")
                    qfs[g] = qf
                    nc.vector.tensor_copy(
                        out=qf, in_=tr_ps.rearrange("m u n -> m (u n)"))

                def phase2(j, last):
                    xpm_j = xpms.pop(j)
                    for cs in range(KT):
                        nc.tensor.matmul(at_ps[:, cs, :], xpm_j[:, cs, :],
                                         kn_all[:, j, :],
                                         start=False, stop=last,
                                         skip_group_check=True)
                    del xpm_j
                    nc.tensor.matmul(ks_ps, ones128, kn_all[:, j, :],
                                     start=False, stop=last,
                                     skip_group_check=True)

                for i in range(NT):
                    if sum(XCHUNKS[:xc]) == i:
                        nt_chunk = XCHUNKS[xc]
                        if xc == 0:
                            x_g = x_first
                        else:
                            x_g = xpool.tile([P, nt_chunk, KT, P], bf16,
                                             name=f"x_{xc}", tag="x")
                            nc.sync.dma_start(out=x_g,
                                              in_=x_ap[:, i:i + nt_chunk])
                        x_base = i
                        xc += 1
                    # late consts: small slices interleaved between x chunks
                    if i in (13, 17, 21, 25):
                        k = (i - 13) // 4
                        nc.sync.dma_start(out=wv_sb[:, k], in_=wv_d.ap()[:, k])
                    x_t = x_g[:, i - x_base]

                    # position-major x^T tiles via PE transposes; two tiles
                    # share one psum bank so the eviction copy is merged
                    if i % 2 == 0:
                        xt_ps = psX_pool.tile([P, 2 * KT, P], bf16,
                                              name=f"xt_{i}", tag="xt")
                    for k in range(KT):
                        nc.tensor.transpose(xt_ps[:, (i % 2) * KT + k, :],
                                            x_t[:, k, :], identb)

                    # Q^T|K^T with biases folded in via K=1 ones-row matmul
                    psA = psA_pool.tile([P, 2 * CQ], f32, name=f"psA_{i}",
                                        tag="psA")
                    for k in range(KT):
                        nc.tensor.matmul(psA, x_t[:, k, :], wqk_sb[:, k, :],
                                         start=(k == 0), stop=False)
                    nc.tensor.matmul(psA, onesrow_b, biasqk,
                                     start=False, stop=True)

                    # phase-2 matmuls, LAG tiles behind (pipeline slack)
                    if len(pending) >= LAG:
                        phase2(pending.pop(0), False)

                    # evictions (xpm: one merged copy per tile pair)
                    if i % 2 == 1:
                        xpm = xpmp.tile([P, 2 * KT, P], bf16,
                                        name=f"xpm_{i}", tag="xpm")
                        xpms[i - 1] = xpm[:, 0:KT]
                        xpms[i] = xpm[:, KT:2 * KT]
                        nc.vector.tensor_copy(out=xpm, in_=xt_ps)
                    sck = scp.tile([P, CQ], f32, name=f"sck_{i}", tag="sck")
                    kss = scp.tile([P, 1], f32, name=f"kss_{i}", tag="kss")
                    nc.scalar.activation(out=sck, in_=psA[:, CQ:2 * CQ],
                                         func=AF.Square, accum_out=kss)
                    nc.scalar.copy(out=qraw_q[i // 8][:, i % 8, 0:CQ],
                                   in_=psA[:, 0:CQ])
                    krt = scp.tile([P, 1], f32, name=f"krt_{i}", tag="krt")
                    nc.scalar.activation(out=krt, in_=kss, func=AF.Sqrt)
                    krs = scp.tile([P, 1], f32, name=f"krs_{i}", tag="krs")
                    nc.vector.reciprocal(out=krs, in_=krt)
                    nc.vector.tensor_scalar_mul(out=kn_all[:, i, 0:CQ],
                                                in0=psA[:, CQ:2 * CQ],
                                                scalar1=krs)
                    pending.append(i)
                    if tq:
                        emit_group_T(tq.pop(0))

                    if i % 8 == 7 and i != NT - 1:
                        h0 = i - 7
                        h1 = i + 1
                        nc.gpsimd.tensor_mul(
                            out=qprod[:, h0:h1, :],
                            in0=qraw_q[i // 8][:, :, 0:CQ],
                            in1=qraw_q[i // 8][:, :, 0:CQ])
                        qsh = q_ss[:, h0:h1]
                        qss3 = bass.AP(tensor=qsh.tensor, offset=qsh.offset,
                                       ap=[qsh.ap[0], qsh.ap[1], [1, 1]])
                        nc.vector.reduce_sum(out=qss3, in_=qprod[:, h0:h1, :],
                                             axis=mybir.AxisListType.X)
                        nc.scalar.activation(out=qrt_all[:, h0:h1],
                                             in_=q_ss[:, h0:h1], func=AF.Sqrt)
                        nc.vector.reciprocal(out=qrs_all[:, h0:h1],
                                             in_=qrt_all[:, h0:h1])
                        nc.vector.tensor_copy(out=qraw_q[i // 8][:, :, CQ],
                                              in_=qrt_all[:, h0:h1])
                        tq.extend([(i // 8) * 2, (i // 8) * 2 + 1])

                # drain remaining phase-2 accumulation
                for idx, j in enumerate(list(pending)):
                    phase2(j, idx == len(pending) - 1)
                while tq:
                    emit_group_T(tq.pop(0))

                # phase-2 results out first: the fold waits on these
                with tc.high_priority():
                    nc.scalar.copy(out=ataug_sb, in_=at_ps)
                    nc.scalar.copy(out=ksrow_sb, in_=ks_ps[0:1, :])
                    nc.vector.tensor_scalar_add(out=ks_eps,
                                                in0=ks_ps[:, 0:CQ],
                                                scalar1=EPS)

            # psAT closed too

            CCH = (0, 4, 8, 16, 24, NT)  # passC chunk boundaries (tiles)

            def passC_mul(h, eng):
                t0, t1 = CCH[h], CCH[h + 1]
                sl3 = (slice(None), slice(t0, t1), slice(None))
                ksb_ap = bass.AP(tensor=ks_eps.tensor, offset=ks_eps.offset,
                                 ap=[ks_eps.ap[0], [0, t1 - t0], [1, CQ]])
                q = t0 // 8
                eng.tensor_mul(
                    out=qprod[sl3],
                    in0=qraw_q[q][:, t0 - 8 * q:t1 - 8 * q, 0:CQ],
                    in1=ksb_ap)

            def passC_rest(h):
                t0, t1 = CCH[h], CCH[h + 1]
                sl = slice(t0, t1)
                sl3 = (slice(None), sl, slice(None))
                qdh = qd[:, sl]
                qd3 = bass.AP(tensor=qdh.tensor, offset=qdh.offset,
                              ap=[qdh.ap[0], qdh.ap[1], [1, 1]])
                nc.vector.reduce_sum(out=qd3, in_=qprod[sl3],
                                     axis=mybir.AxisListType.X)
                nc.vector.tensor_mul(out=dn[:, sl], in0=qd[:, sl],
                                     in1=qrs_all[:, sl])
                nc.vector.tensor_scalar_add(out=dn[:, sl], in0=dn[:, sl],
                                            scalar1=float(N))
                nc.vector.reciprocal(out=tailor[:, sl], in_=dn[:, sl])
                nc.vector.tensor_mul(out=s_all[:, sl], in0=tailor[:, sl],
                                     in1=qrs_all[:, sl])

            with tc.tile_pool(name="psM", bufs=1, space="PSUM") as psM_pool:
                # fold M1_aug = A_aug @ Wv^T + [ksum|N] (x) bv  (Wv, bv carry
                # the gamma factor, folded on the host)
                with tc.high_priority():
                    m1_ps = psM_pool.tile([CQ + 1, C], f32)
                    for k in range(KT):
                        nc.tensor.matmul(m1_ps, ataug_sb[:, k, :],
                                         wv_sb[:, k, :],
                                         start=(k == 0), stop=False)
                    nc.tensor.matmul(m1_ps, ksrow_sb, bvrow,
                                     start=False, stop=True)
                    nc.scalar.copy(out=m1_sb, in_=m1_ps)

            with tc.tile_pool(name="psE", bufs=6, space="PSUM") as psE_pool:
                with tc.high_priority():
                    passC_mul(0, nc.vector)
                    passC_rest(0)
                    passC_mul(1, nc.vector)
                    passC_rest(1)
                passC_mul(2, nc.gpsimd)

                def quarter3():
                    h0, h1 = NT - 8, NT
                    nc.gpsimd.tensor_mul(out=qprod[:, h0:h1, :],
                                         in0=qraw_q[3][:, :, 0:CQ],
                                         in1=qraw_q[3][:, :, 0:CQ])
                    qsh = q_ss[:, h0:h1]
                    qss3 = bass.AP(tensor=qsh.tensor, offset=qsh.offset,
                                   ap=[qsh.ap[0], qsh.ap[1], [1, 1]])
                    nc.vector.reduce_sum(out=qss3, in_=qprod[:, h0:h1, :],
                                         axis=mybir.AxisListType.X)
                    nc.scalar.activation(out=qrt_all[:, h0:h1],
                                         in_=q_ss[:, h0:h1], func=AF.Sqrt)
                    nc.vector.reciprocal(out=qrs_all[:, h0:h1],
                                         in_=qrt_all[:, h0:h1])
                    nc.vector.tensor_copy(out=qraw_q[3][:, :, CQ],
                                          in_=qrt_all[:, h0:h1])

                # --- pass E: out^T [n, c] = qf_raw_chunk^T @ M1_aug; the
                # per-position scale s = tailor/||Q|| is applied at eviction
                # as a per-partition scalar (ACT Copy-with-scale / DVE).
                NG = NCHUNK // P  # 4 tiles per group
                NGRP = NT // NG
                for g in range(NGRP):
                    if g == 1:
                        quarter3()
                        passC_rest(2)
                        passC_mul(3, nc.gpsimd)
                        passC_mul(4, nc.gpsimd)
                    if g == 2:
                        emit_group_T(6)
                    if g == 3:
                        emit_group_T(7)
                        passC_rest(3)
                    if g == 5:
                        passC_rest(4)
                    qf = qfs.pop(g)
                    obuf = obp.tile([P, NG, C], bf16, name=f"ob_{g}",
                                    tag="ob")
                    for u in range(NG):
                        i = g * NG + u
                        out_ps = psE_pool.tile([P, C], f32,
                                               name=f"ops_{u}_{g}",
                                               tag="ops")
                        nc.tensor.matmul(out_ps, qf[:, u * P:(u + 1) * P],
                                         m1_sb, start=True, stop=True)
                        s_ap = s_all[:, i:i + 1]
                        if i % 3 == 1:
                            nc.vector.tensor_scalar_mul(
                                out=obuf[:, u, :], in0=out_ps, scalar1=s_ap)
                        else:
                            nc.scalar.activation(
                                out=obuf[:, u, :], in_=out_ps,
                                func=AF.Copy, scale=s_ap)
                    nc.sync.dma_start(
                        out=out_d.ap()[g * NCHUNK:(g + 1) * NCHUNK, :]
                        .rearrange("(u p) c -> p u c", p=P),
                        in_=obuf)

    nc.compile()
    return nc


def _get_nc():
    if "nc" not in _CACHE:
        _CACHE["nc"] = _build()
    return _CACHE["nc"]


def _prep_inputs(x, Wq, bq, Wk, bk, Wv, bv, gamma):
    x = np.ascontiguousarray(np.asarray(x, dtype=np.float32)).reshape(B, C, N)
    # channel-major bf16: xh[b, p, i, k, j] = x[b, k*128+p, i*128+j]
    xh = np.ascontiguousarray(
        x.reshape(B, KT, P, NT, P).transpose(0, 2, 3, 1, 4)).astype(BF16)
    wqk = np.concatenate([np.asarray(Wq, np.float32).T,
                          np.asarray(Wk, np.float32).T], axis=1)  # [C, 128]
    wqk = np.ascontiguousarray(
        wqk.reshape(KT, P, 2 * CQ).transpose(1, 0, 2)).astype(BF16)
    g = np.float32(np.asarray(gamma).reshape(-1)[0])
    wvt = np.asarray(Wv, np.float32).T * g        # [C(c'), C], gamma folded
    wvt = np.ascontiguousarray(
        wvt.reshape(KT, P, C).transpose(1, 0, 2)).astype(BF16)

    blobb = np.zeros((P, _BLOBB_W), np.float32)
    blobb[:, 0:P] = np.eye(P)
    blobb[:, _OCOL] = 1.0
    blobb[:, _O32:_O32 + NT] = 1.0
    blobb[0, _OBIAS:_OBIAS + CQ] = np.asarray(bq, np.float32)
    blobb[0, _OBIAS + CQ:_OBIAS + 2 * CQ] = np.asarray(bk, np.float32)
    blobb[0, _OROW:_OROW + P] = 1.0
    blobb[:, _OONE:_OONE + P] = 1.0
    blobb[0, _OBV:_OBV + C] = np.asarray(bv, np.float32) * g

    return {
        "x": xh,
        "wqk": wqk,
        "wv": wvt,
        "blobb": blobb.astype(BF16),
    }


def kernel(x, Wq, bq, Wk, bk, Wv, bv, gamma, _trace=False):
    from concourse.bass_utils import run_bass_kernel_spmd

    common = _prep_inputs(x, Wq, bq, Wk, bk, Wv, bv, gamma)
    xh = common.pop("x")
    nc = _get_nc()
    in_maps = [{"x": xh[i], **common} for i in range(B)]
    res = run_bass_kernel_spmd(nc, in_maps, list(range(B)), trace=_trace)
    out = np.stack([np.asarray(res.results[i]["out"]).astype(np.float32).T
                    for i in range(B)])
    if _trace:
        _CACHE["last_results"] = res
    return np.ascontiguousarray(out).reshape(B, C, H, W)
